# revision 8
# baseline (speedup 1.0000x reference)
"""Trainium2 Bass kernel for nn_DeepCluster (vq_codebook).

Computation (per row x of shape [72]):
  7-layer MLP (72-128-256-256-512-512-512-200, ReLU after layers 2/4) -> f
  sq[j]  = |f|^2 - 2*(f @ center)[j] + |center[:, j]|^2      (center: [200, 72])
  q      = (1/(1 + sq)) / sum_j (1/(1 + sq))                  (alpha = 1)

Strategy: pure data parallel over 8 NeuronCores (batch split).  Activations
flow as [features(partitions), batch(free)] tiles; the input is transposed
host-side so no on-device input transposes are needed.  Layers 3..7 run in
fp8(e4m3) with DoubleRow matmuls (2 fp8 weights per PE cell -> 2 MACs/cell/
cycle), which roughly halves PE time for 94% of the MACs.  All fp8 scaling
uses power-of-2 factors folded into the PSUM->SBUF epilogues (one mult+add
or add+max per drain, so every drain fits either the Scalar(ACT) or
Vector(DVE) engine).  Each drain covers a [*, 2, 512] PSUM pair (1024 cols)
to amortize fixed PSUM-access overheads.  |center_j|^2 + 1 rides into the
distance matmul through two constant-one rows appended to the f1 tile
(bf16 hi+lo split for precision), eliminating a separate csq pass.  The
per-tile tail (reciprocal -> transpose back -> row-normalize -> store) is
software-pipelined one tile behind the MLP so the PE never waits on it.
"""

import numpy as np

DIMS = [72, 128, 256, 256, 512, 512, 512, 200]
N_CORES = 8
SUB = 512     # moving free-dim per matmul (one PSUM bank of fp32)
NSUB = 2      # subs per pipeline tile
T = SUB * NSUB
P = 128

# fp8 scaling plan (all powers of two; folded into epilogues):
#   h1 stored bf16 at 8x (w1 pre-scaled by 8)
#   psum2 = (8 h1)(8 w2) = 64*pre2 ; h2 = relu(psum2 + 64 b2)      [A2 = 64]
#   L3: S3=2048 ; drain scale 64/(64*2048) = 2^-11, bias 64 b3     [A3 = 64]
#   L4: S4=1    ; drain add 64 b4, max 0                           [A4 = 64]
#   L5: S5=4096 ; scale 256/(64*4096) = 2^-10, bias 256 b5         [A5 = 256]
#   L6: S6=4096 ; scale 256/(256*4096) = 2^-12, bias 256 b6        [A6 = 256]
#   L7: S7=4096 ; scale 1/(256*4096) = 2^-20, bias b7 -> f (bf16, true scale)
W1S = 8.0
W2S = 8.0
S_W = {3: 2048.0, 4: 1.0, 5: 4096.0, 6: 4096.0, 7: 4096.0}
A_ACT = {1: 8.0, 2: 64.0, 3: 64.0, 4: 64.0, 5: 256.0, 6: 256.0, 7: 1.0}
# drain (scale, relu) per layer; scale==1.0 with relu -> add+max (DVE ok)
DRAIN_SCALE = {1: 1.0, 2: 1.0, 3: 2.0 ** -11, 4: 1.0,
               5: 2.0 ** -10, 6: 2.0 ** -12, 7: 2.0 ** -20}
RELU_LAYERS = {2, 4}
# ACT share of PSUM drains (ACT is a bit faster per column; DVE also owns
# the distance tail).  Engines are assigned in emission order via a
# Bresenham accumulator so consecutive drains alternate A/V even with two
# tiles interleaved — same-engine runs would stall the PE on PSUM reuse.
ACT_DRAIN_SHARE = 12.0 / 19.0
W7PAD = 256  # L7 out cols padded 200 -> 256 for 16B-aligned DR weight APs

_CACHE = {}


def _build(n_loc):
    import concourse.bass as bass
    import concourse.mybir as mybir
    from concourse import bacc
    from concourse.tile import TileContext
    from concourse.masks import make_identity

    f32 = mybir.dt.float32
    bf16 = mybir.dt.bfloat16
    f8 = mybir.dt.float8e4
    AF = mybir.ActivationFunctionType
    AX = mybir.AxisListType
    ALU = mybir.AluOpType
    DR = mybir.MatmulPerfMode.DoubleRow

    nc = bacc.Bacc(None, target_bir_lowering=False, debug=False)
    x_d = nc.dram_tensor("x", [72, n_loc], bf16, kind="ExternalInput")
    q_d = nc.dram_tensor("q", [n_loc, 72], f32, kind="ExternalOutput")
    w_d = [
        nc.dram_tensor("w1", [72, 128], bf16, kind="ExternalInput"),
        nc.dram_tensor("w2", [128, 256], bf16, kind="ExternalInput"),
        nc.dram_tensor("w3", [128, 2, 256], f8, kind="ExternalInput"),
        nc.dram_tensor("w4", [128, 2, 512], f8, kind="ExternalInput"),
        nc.dram_tensor("w5", [128, 4, 512], f8, kind="ExternalInput"),
        nc.dram_tensor("w6", [128, 4, 512], f8, kind="ExternalInput"),
        nc.dram_tensor("w7", [128, 4, W7PAD], f8, kind="ExternalInput"),
    ]
    mc_l = [1, 2, 2, 4, 4, 4, 2]
    kp_l = [1, 1, 1, 1, 2, 2, 2]  # DoubleRow k-pairs for fp8 layers (L3+)
    b_d = [nc.dram_tensor(f"b{l + 1}", [128, mc_l[l]], f32, kind="ExternalInput")
           for l in range(7)]
    cm2A_d = nc.dram_tensor("cm2A", [128, 72], bf16, kind="ExternalInput")
    cm2B_d = nc.dram_tensor("cm2B", [74, 72], bf16, kind="ExternalInput")

    NT = n_loc // T
    assert n_loc % T == 0

    with TileContext(nc) as tc:
        with (
            tc.tile_pool(name="consts", bufs=1) as consts,
            tc.tile_pool(name="acts", bufs=3) as acts,
            tc.tile_pool(name="pmm", bufs=2, space="PSUM") as pmm,
            tc.tile_pool(name="psd", bufs=2, space="PSUM") as psd,
            tc.tile_pool(name="ppq", bufs=2, space="PSUM") as ppq,
        ):
            ones = consts.tile([128, 72], bf16, tag="ones")
            nc.vector.memset(ones, 1.0)
            identf = consts.tile([128, 128], f32, tag="identf")
            make_identity(nc, identf)
            cm2A = consts.tile([128, 72], bf16, tag="cm2A")
            nc.sync.dma_start(out=cm2A, in_=cm2A_d[:])
            cm2B = consts.tile([74, 72], bf16, tag="cm2B")
            nc.sync.dma_start(out=cm2B, in_=cm2B_d[:])
            w_sb, b_sb = [], []
            for l in range(7):
                dt_w = bf16 if l < 2 else f8
                wt = consts.tile(list(w_d[l].shape), dt_w, tag=f"w{l}")
                nc.sync.dma_start(out=wt, in_=w_d[l][:])
                w_sb.append(wt)
                bt = consts.tile([128, mc_l[l]], f32, tag=f"bias{l}")
                nc.sync.dma_start(out=bt, in_=b_d[l][:])
                b_sb.append(bt)

            x_r = x_d[:].rearrange("j (t u c) -> t j u c", u=NSUB, c=SUB)
            q_r = q_d[:].rearrange("(t u s p) j -> t u p s j", u=NSUB, s=T // NSUB // P, p=P)

            def prefetch(t):
                xt = acts.tile([72, NSUB, SUB], bf16, tag="xT")
                nc.sync.dma_start(out=xt, in_=x_r[t])
                return xt

            dracc = [0.0]

            def drain(l, m, ps_sub, out_ap):
                """PSUM pair [pw, NSUB, SUB] -> SBUF with scale/bias/relu."""
                dracc[0] += ACT_DRAIN_SHARE
                if dracc[0] >= 1.0:
                    dracc[0] -= 1.0
                    eng = "A"
                else:
                    eng = "V"
                sc = DRAIN_SCALE[l]
                relu = l in RELU_LAYERS
                bias_col = b_sb[l - 1][: ps_sub.shape[0], m : m + 1]
                if eng == "A":
                    nc.scalar.activation(
                        out=out_ap, in_=ps_sub,
                        func=AF.Relu if relu else AF.Identity,
                        bias=bias_col, scale=sc,
                    )
                elif relu:
                    assert sc == 1.0
                    nc.vector.tensor_scalar(
                        out=out_ap, in0=ps_sub, scalar1=bias_col, scalar2=0.0,
                        op0=ALU.add, op1=ALU.max,
                    )
                elif sc == 1.0:
                    nc.vector.tensor_scalar_add(out_ap, ps_sub, bias_col)
                else:
                    nc.vector.tensor_scalar(
                        out=out_ap, in0=ps_sub, scalar1=sc, scalar2=bias_col,
                        op0=ALU.mult, op1=ALU.add,
                    )

            def dist_pe(rec):
                """Distance matmuls for a finished tile -> psd PSUM tiles."""
                sds = []
                for s in range(NSUB):
                    sd = psd.tile([72, SUB], f32, tag="sd")
                    nc.tensor.matmul(sd, ones, rec["g0"][:, s, :],
                                     start=True, stop=False)
                    nc.tensor.matmul(sd, ones[:72, :72], rec["g1"][:, s, :],
                                     start=False, stop=False)
                    nc.tensor.matmul(sd, cm2A, rec["f0"][:, s, :],
                                     start=False, stop=False)
                    nc.tensor.matmul(sd, cm2B, rec["f1"][:, s, :],
                                     start=False, stop=True)
                    sds.append(sd)
                rec["sd"] = sds

            def recip(rec):
                nomT = acts.tile([72, NSUB, SUB], f32, tag="nomT")
                for s in range(NSUB):
                    nc.vector.reciprocal_approx_fast(
                        out=nomT[:, s, :], in_=rec["sd"][s])
                rec["nomT"] = nomT

            def tail(rec):
                """Transpose back, row-normalize, store (one sub at a time)."""
                nomT = rec["nomT"]
                C = T // NSUB // P
                for s in range(NSUB):
                    pq = ppq.tile([P, C, 72], f32, tag="pq")
                    for c in range(C):
                        nc.tensor.transpose(
                            pq[:, c, :], nomT[:, s, P * c : P * (c + 1)],
                            identf[:72, :72])
                    rs = acts.tile([P, C], f32, tag="rs")
                    nc.vector.reduce_sum(rs, pq, axis=AX.X)
                    rr = acts.tile([P, C], f32, tag="rr")
                    nc.vector.reciprocal(rr, rs)
                    rr_b = bass.AP(
                        tensor=rr.tensor, offset=rr.offset,
                        ap=[rr.ap[0], rr.ap[1], [0, 72]],
                    )
                    qt = acts.tile([P, C, 72], f32, tag="qt")
                    nc.vector.tensor_tensor(out=qt, in0=pq, in1=rr_b, op=ALU.mult)
                    nc.sync.dma_start(out=q_r[rec["t"], s], in_=qt)

            def mm_layer(l, h_in, kc_in, out_tile, out_l):
                """Emit layer l's matmuls + drains.  h_in: [*, kc_in, NSUB, SUB]
                (or xT [72, NSUB, SUB] for l=1).  Writes out_tile[:, m, :, :]."""
                dout = DIMS[l]
                mc = mc_l[l - 1]
                for m in range(mc):
                    pw = min(128, dout - 128 * m)
                    ps = pmm.tile([128, NSUB, SUB], f32, tag="mm")
                    for s in range(NSUB):
                        if l == 1:
                            nc.tensor.matmul(ps[:, s, :], w_sb[0],
                                             h_in[:, s, :], start=True, stop=True)
                        elif l == 2:
                            nc.tensor.matmul(
                                ps[:, s, :],
                                w_sb[1][:, 128 * m : 128 * m + pw],
                                h_in[:, 0, s, :], start=True, stop=True)
                        else:
                            kp = kp_l[l - 1]
                            for j in range(kp):
                                nc.tensor.matmul(
                                    ps[:pw, s, :],
                                    w_sb[l - 1][:, 2 * j : 2 * j + 2,
                                                128 * m : 128 * m + pw],
                                    h_in[:, 2 * j : 2 * j + 2, s, :],
                                    start=(j == 0), stop=(j == kp - 1),
                                    perf_mode=DR)
                    if out_l == "f1":
                        drain(l, m, ps[:pw], out_tile[0:72])
                    elif out_l == "f0":
                        drain(l, m, ps[:pw], out_tile)
                    else:
                        drain(l, m, ps[:pw], out_tile[:pw, m])

            def alloc_rec(t, xT):
                rec = {"t": t, "xT": xT}
                rec["f1"] = acts.tile([74, NSUB, SUB], bf16, tag=f"f1{t % 2}", name=f"f1_{t % 2}")
                # Pool-engine partition access must start 32-aligned; rows
                # 64:72 are re-written by the L7 m=1 drain below.
                nc.gpsimd.memset(rec["f1"][64:74], 1.0)
                for nm, kc, dt_h in (("h1", 1, bf16), ("h2", 2, f8),
                                     ("h3", 2, f8), ("h4", 4, f8),
                                     ("h5", 4, f8), ("h6", 4, f8)):
                    rec[nm] = acts.tile([128, kc, NSUB, SUB], dt_h,
                                        tag=f"{nm}{t % 2}", name=f"{nm}_{t % 2}")
                rec["f0"] = acts.tile([128, NSUB, SUB], bf16, tag=f"f0{t % 2}", name=f"f0_{t % 2}")
                return rec

            def layer(l, rec):
                if l < 7:
                    h_in = rec["xT"] if l == 1 else rec[f"h{l - 1}"]
                    mm_layer(l, h_in, None, rec[f"h{l}"], "h")
                    return
                # L7: m=0 -> f0, m=1 -> f1 rows 0:72
                dout, mc = DIMS[7], mc_l[6]
                for m in range(mc):
                    pw = min(128, dout - 128 * m)
                    ps = pmm.tile([128, NSUB, SUB], f32, tag="mm")
                    for s in range(NSUB):
                        for j in range(2):
                            nc.tensor.matmul(
                                ps[:pw, s, :],
                                w_sb[6][:, 2 * j : 2 * j + 2,
                                        128 * m : 128 * m + pw],
                                rec["h6"][:, 2 * j : 2 * j + 2, s, :],
                                start=(j == 0), stop=(j == 1), perf_mode=DR)
                    drain(7, m, ps[:pw], rec["f0"] if m == 0 else rec["f1"][0:72])

            def squares(rec):
                g0 = acts.tile([128, NSUB, SUB], bf16, tag=f"g0{rec['t'] % 2}")
                nc.gpsimd.tensor_tensor(out=g0, in0=rec["f0"], in1=rec["f0"],
                                        op=ALU.mult)
                g1 = acts.tile([72, NSUB, SUB], bf16, tag=f"g1{rec['t'] % 2}")
                nc.gpsimd.tensor_tensor(out=g1, in0=rec["f1"][0:72],
                                        in1=rec["f1"][0:72], op=ALU.mult)
                rec.update(g0=g0, g1=g1)

            # Two tiles are emitted interleaved layer-by-layer so one tile's
            # matmuls hide the other's PSUM-drain latency; the previous
            # pair's distance/normalize work is threaded between layers as
            # PE/DVE filler.
            assert NT % 2 == 0
            xts = [prefetch(0), prefetch(1)]
            prev = []
            for tp in range(0, NT, 2):
                a = alloc_rec(tp, xts[0])
                b = alloc_rec(tp + 1, xts[1])
                layer(1, a)
                layer(1, b)
                if tp + 2 < NT:
                    xts = [prefetch(tp + 2), prefetch(tp + 3)]
                if prev:
                    dist_pe(prev[0])
                    recip(prev[0])
                layer(2, a)
                layer(2, b)
                if prev:
                    dist_pe(prev[1])
                    recip(prev[1])
                layer(3, a)
                layer(3, b)
                layer(4, a)
                layer(4, b)
                if prev:
                    tail(prev[0])
                layer(5, a)
                layer(5, b)
                if prev:
                    tail(prev[1])
                layer(6, a)
                layer(6, b)
                layer(7, a)
                layer(7, b)
                squares(a)
                squares(b)
                prev = [a, b]
            for r in prev:
                dist_pe(r)
                recip(r)
                tail(r)

    nc.compile()
    return nc


def _prep_consts(ws, bs, center):
    """Host-side marshalling of the small replicated weights."""
    import ml_dtypes

    bf = ml_dtypes.bfloat16
    f8 = ml_dtypes.float8_e4m3
    consts = {}

    def q8(w, s):
        return np.clip(np.asarray(w, np.float32) * s, -240.0, 240.0).astype(f8)

    consts["w1"] = np.ascontiguousarray(np.asarray(ws[0], np.float32) * W1S).astype(bf)
    consts["w2"] = np.ascontiguousarray(np.asarray(ws[1], np.float32) * W2S).astype(bf)
    for l in range(3, 8):
        w = q8(ws[l - 1], S_W[l])  # [din, dout]
        din, dout = w.shape
        kc = din // 128
        if l == 7:
            wp = np.zeros((din, W7PAD), dtype=f8)
            wp[:, :dout] = w
            w, dout = wp, W7PAD
        consts[f"w{l}"] = np.ascontiguousarray(
            w.reshape(kc, 128, dout).transpose(1, 0, 2))
    mc_l = [1, 2, 2, 4, 4, 4, 2]
    for l in range(1, 8):
        dout = DIMS[l]
        bt = np.zeros((128, mc_l[l - 1]), dtype=np.float32)
        bias = np.asarray(bs[l - 1], np.float32) * A_ACT[l]
        for m in range(mc_l[l - 1]):
            pw = min(128, dout - 128 * m)
            bt[:pw, m] = bias[128 * m : 128 * m + pw]
        consts[f"b{l}"] = bt
    c = np.asarray(center, np.float64)
    consts["cm2A"] = np.ascontiguousarray(-2.0 * c[:128, :]).astype(np.float32).astype(bf)
    cm2B = np.zeros((74, 72), dtype=np.float32)
    cm2B[:72] = (-2.0 * c[128:200, :]).astype(np.float32)
    csq = (1.0 + (c ** 2).sum(axis=0)).astype(np.float32)
    csq_hi = csq.astype(bf).astype(np.float32)
    cm2B[72] = csq_hi          # hi part of csq (bf16-rounded)
    cm2B[73] = csq - csq_hi    # lo correction, small enough for bf16
    consts["cm2B"] = np.ascontiguousarray(cm2B).astype(bf)
    return consts


def _prep_inputs(inputs):
    """Full input dict -> per-core input maps (x transposed host-side)."""
    import ml_dtypes

    x = np.asarray(inputs["inputs"], np.float32)
    n = x.shape[0]
    n_loc = n // N_CORES
    xt = np.ascontiguousarray(x.T.astype(ml_dtypes.bfloat16))  # [72, N]
    consts = _prep_consts(
        [inputs[f"w{i}"] for i in range(1, 8)],
        [inputs[f"b{i}"] for i in range(1, 8)],
        inputs["center"],
    )
    in_maps = []
    for c in range(N_CORES):
        m = {"x": np.ascontiguousarray(xt[:, c * n_loc : (c + 1) * n_loc])}
        m.update(consts)
        in_maps.append(m)
    return in_maps, n_loc


def kernel(
    inputs, w1, b1, w2, b2, w3, b3, w4, b4, w5, b5, w6, b6, w7, b7, center
):
    from concourse.bass_utils import run_bass_kernel_spmd

    full = dict(inputs=inputs, w1=w1, b1=b1, w2=w2, b2=b2, w3=w3, b3=b3,
                w4=w4, b4=b4, w5=w5, b5=b5, w6=w6, b6=b6, w7=w7, b7=b7,
                center=center)
    in_maps, n_loc = _prep_inputs(full)
    if n_loc not in _CACHE:
        _CACHE[n_loc] = _build(n_loc)
    nc = _CACHE[n_loc]
    res = run_bass_kernel_spmd(nc, in_maps, core_ids=list(range(N_CORES)))
    return np.concatenate([res.results[c]["q"] for c in range(N_CORES)], axis=0)


# revision 12
# speedup vs baseline: 1.2472x; 1.2472x over previous
"""Trainium2 Bass kernel for nn_DeepCluster (vq_codebook).

Computation (per row x of shape [72]):
  7-layer MLP (72-128-256-256-512-512-512-200, ReLU after layers 2/4) -> f
  sq[j]  = |f|^2 - 2*(f @ center)[j] + |center[:, j]|^2      (center: [200, 72])
  q      = (1/(1 + sq)) / sum_j (1/(1 + sq))                  (alpha = 1)

Strategy: pure data parallel over 8 NeuronCores (batch split).  Activations
flow as [features(partitions), batch(free)] tiles; the input is transposed
host-side so no on-device input transposes are needed.  Layers 3..7 run in
fp8(e4m3) with DoubleRow matmuls (2 fp8 weights per PE cell -> 2 MACs/cell/
cycle), which roughly halves PE time for 94% of the MACs.  All fp8 scaling
uses power-of-2 factors folded into the PSUM->SBUF epilogues (one mult+add
or add+max per drain, so every drain fits either the Scalar(ACT) or
Vector(DVE) engine).  Each drain covers a [*, 2, 512] PSUM pair (1024 cols)
to amortize fixed PSUM-access overheads.  |center_j|^2 + 1 rides into the
distance matmul through two constant-one rows appended to the f1 tile
(bf16 hi+lo split for precision), eliminating a separate csq pass.  The
per-tile tail (reciprocal -> transpose back -> row-normalize -> store) is
software-pipelined one tile behind the MLP so the PE never waits on it.
"""

import numpy as np

DIMS = [72, 128, 256, 256, 512, 512, 512, 200]
N_CORES = 8
SUB = 512     # moving free-dim per matmul (one PSUM bank of fp32)
NSUB = 2      # subs per pipeline tile
T = SUB * NSUB
P = 128

# fp8 scaling plan (all powers of two; folded into epilogues):
#   h1 stored bf16 at 8x (w1 pre-scaled by 8)
#   psum2 = (8 h1)(8 w2) = 64*pre2 ; h2 = relu(psum2 + 64 b2)      [A2 = 64]
#   L3: S3=2048 ; drain scale 64/(64*2048) = 2^-11, bias 64 b3     [A3 = 64]
#   L4: S4=1    ; drain add 64 b4, max 0                           [A4 = 64]
#   L5: S5=4096 ; scale 256/(64*4096) = 2^-10, bias 256 b5         [A5 = 256]
#   L6: S6=4096 ; scale 256/(256*4096) = 2^-12, bias 256 b6        [A6 = 256]
#   L7: S7=4096 ; scale 1/(256*4096) = 2^-20, bias b7 -> f (bf16, true scale)
W1S = 8.0
W2S = 8.0
S_W = {3: 2048.0, 4: 1.0, 5: 4096.0, 6: 4096.0, 7: 4096.0}
A_ACT = {1: 8.0, 2: 64.0, 3: 64.0, 4: 64.0, 5: 256.0, 6: 256.0, 7: 1.0}
# drain (scale, relu) per layer; scale==1.0 with relu -> add+max (DVE ok)
DRAIN_SCALE = {1: 1.0, 2: 1.0, 3: 2.0 ** -11, 4: 1.0,
               5: 2.0 ** -10, 6: 2.0 ** -12, 7: 2.0 ** -20}
RELU_LAYERS = {2, 4}
# ACT share of PSUM drains (ACT is a bit faster per column; DVE also owns
# the distance tail).  Engines are assigned in emission order via a
# Bresenham accumulator so consecutive drains alternate A/V even with two
# tiles interleaved — same-engine runs would stall the PE on PSUM reuse.
ACT_DRAIN_SHARE = 12.0 / 19.0
W7PAD = 256  # L7 out cols padded 200 -> 256 for 16B-aligned DR weight APs

_CACHE = {}


def _build(n_loc):
    import concourse.bass as bass
    import concourse.mybir as mybir
    from concourse import bacc
    from concourse.tile import TileContext
    from concourse.masks import make_identity

    f32 = mybir.dt.float32
    bf16 = mybir.dt.bfloat16
    f8 = mybir.dt.float8e4
    AF = mybir.ActivationFunctionType
    AX = mybir.AxisListType
    ALU = mybir.AluOpType
    DR = mybir.MatmulPerfMode.DoubleRow

    nc = bacc.Bacc(None, target_bir_lowering=False, debug=False)
    x_d = nc.dram_tensor("x", [72, n_loc], bf16, kind="ExternalInput")
    q_d = nc.dram_tensor("q", [n_loc, 72], f32, kind="ExternalOutput")
    w_d = [
        nc.dram_tensor("w1", [72, 128], bf16, kind="ExternalInput"),
        nc.dram_tensor("w2", [128, 256], bf16, kind="ExternalInput"),
        nc.dram_tensor("w3", [128, 2, 256], f8, kind="ExternalInput"),
        nc.dram_tensor("w4", [128, 2, 512], f8, kind="ExternalInput"),
        nc.dram_tensor("w5", [128, 4, 512], f8, kind="ExternalInput"),
        nc.dram_tensor("w6", [128, 4, 512], f8, kind="ExternalInput"),
        nc.dram_tensor("w7", [128, 4, W7PAD], f8, kind="ExternalInput"),
    ]
    mc_l = [1, 2, 2, 4, 4, 4, 2]
    kp_l = [1, 1, 1, 1, 2, 2, 2]  # DoubleRow k-pairs for fp8 layers (L3+)
    b_d = [nc.dram_tensor(f"b{l + 1}", [128, mc_l[l]], f32, kind="ExternalInput")
           for l in range(7)]
    cm2A_d = nc.dram_tensor("cm2A", [128, 72], bf16, kind="ExternalInput")
    cm2B_d = nc.dram_tensor("cm2B", [74, 72], bf16, kind="ExternalInput")

    NT = n_loc // T
    assert n_loc % T == 0

    with TileContext(nc) as tc:
        with (
            tc.tile_pool(name="consts", bufs=1) as consts,
            tc.tile_pool(name="acts", bufs=3) as acts,
            tc.tile_pool(name="pmm", bufs=3, space="PSUM") as pmm,
            tc.tile_pool(name="psdq", bufs=2, space="PSUM") as psdq,
        ):
            ones = consts.tile([128, 72], bf16, tag="ones")
            nc.vector.memset(ones, 1.0)
            identf = consts.tile([128, 128], f32, tag="identf")
            make_identity(nc, identf)
            cm2A = consts.tile([128, 72], bf16, tag="cm2A")
            nc.sync.dma_start(out=cm2A, in_=cm2A_d[:])
            cm2B = consts.tile([74, 72], bf16, tag="cm2B")
            nc.sync.dma_start(out=cm2B, in_=cm2B_d[:])
            w_sb, b_sb = [], []
            for l in range(7):
                dt_w = bf16 if l < 2 else f8
                wt = consts.tile(list(w_d[l].shape), dt_w, tag=f"w{l}")
                nc.sync.dma_start(out=wt, in_=w_d[l][:])
                w_sb.append(wt)
                bt = consts.tile([128, mc_l[l]], f32, tag=f"bias{l}")
                nc.sync.dma_start(out=bt, in_=b_d[l][:])
                b_sb.append(bt)

            x_r = x_d[:].rearrange("j (t u c) -> t j u c", u=NSUB, c=SUB)
            q_r = q_d[:].rearrange("(t u s p) j -> t u p s j", u=NSUB, s=T // NSUB // P, p=P)

            def prefetch(t):
                xt = acts.tile([72, NSUB, SUB], bf16, tag="xT")
                nc.sync.dma_start(out=xt, in_=x_r[t])
                return xt

            dracc = [0.0]

            def drain(l, m, ps_sub, out_ap):
                """PSUM pair [pw, NSUB, SUB] -> SBUF with scale/bias/relu."""
                dracc[0] += ACT_DRAIN_SHARE
                if dracc[0] >= 1.0:
                    dracc[0] -= 1.0
                    eng = "A"
                else:
                    eng = "V"
                sc = DRAIN_SCALE[l]
                relu = l in RELU_LAYERS
                bias_col = b_sb[l - 1][: ps_sub.shape[0], m : m + 1]
                if eng == "A":
                    nc.scalar.activation(
                        out=out_ap, in_=ps_sub,
                        func=AF.Relu if relu else AF.Identity,
                        bias=bias_col, scale=sc,
                    )
                elif relu:
                    assert sc == 1.0
                    nc.vector.tensor_scalar(
                        out=out_ap, in0=ps_sub, scalar1=bias_col, scalar2=0.0,
                        op0=ALU.add, op1=ALU.max,
                    )
                elif sc == 1.0:
                    nc.vector.tensor_scalar_add(out_ap, ps_sub, bias_col)
                else:
                    nc.vector.tensor_scalar(
                        out=out_ap, in0=ps_sub, scalar1=sc, scalar2=bias_col,
                        op0=ALU.mult, op1=ALU.add,
                    )

            def dist_pe(rec):
                """Distance matmuls for a finished tile.  Each (tile, sub)
                gets one [128, 512] PSUM bank: rows 0:72 hold sd; after the
                reciprocal consumes it, the same bank takes the transposed-
                back nom chunks (the two uses are disjoint in time)."""
                sds = []
                for s in range(NSUB):
                    sdq = psdq.tile([128, SUB], f32, tag="sdq", name=f"sdq{s}")
                    sd = sdq[:72, :]
                    nc.tensor.matmul(sd, ones, rec["g0"][:, s, :],
                                     start=True, stop=False)
                    nc.tensor.matmul(sd, ones[:72, :72], rec["g1"][:, s, :],
                                     start=False, stop=False)
                    nc.tensor.matmul(sd, cm2A, rec["f0"][:, s, :],
                                     start=False, stop=False)
                    nc.tensor.matmul(sd, cm2B, rec["f1"][:, s, :],
                                     start=False, stop=True)
                    sds.append(sdq)
                rec["sdq"] = sds

            def recip(rec):
                nomT = acts.tile([72, NSUB, SUB], f32, tag="nomT")
                for s in range(NSUB):
                    nc.vector.reciprocal_approx_fast(
                        out=nomT[:, s, :], in_=rec["sdq"][s][:72, :])
                rec["nomT"] = nomT

            def tail(rec):
                """Transpose back, row-normalize, store (one sub at a time)."""
                nomT = rec["nomT"]
                C = T // NSUB // P
                for s in range(NSUB):
                    sdq = rec["sdq"][s]
                    for c in range(C):
                        nc.tensor.transpose(
                            sdq[:, 72 * c : 72 * (c + 1)],
                            nomT[:, s, P * c : P * (c + 1)],
                            identf[:72, :72])
                    pq = bass.AP(tensor=sdq.tensor, offset=sdq.offset,
                                 ap=[sdq.ap[0], [72, C], [1, 72]])
                    rs = acts.tile([P, C], f32, tag="rs")
                    nc.vector.reduce_sum(rs, pq, axis=AX.X)
                    rr = acts.tile([P, C], f32, tag="rr")
                    nc.vector.reciprocal(rr, rs)
                    rr_b = bass.AP(
                        tensor=rr.tensor, offset=rr.offset,
                        ap=[rr.ap[0], rr.ap[1], [0, 72]],
                    )
                    qt = acts.tile([P, C, 72], f32, tag="qt")
                    nc.vector.tensor_tensor(out=qt, in0=pq, in1=rr_b, op=ALU.mult)
                    nc.sync.dma_start(out=q_r[rec["t"], s], in_=qt)

            def mm_layer(l, h_in, kc_in, out_tile, out_l):
                """Emit layer l's matmuls + drains.  h_in: [*, kc_in, NSUB, SUB]
                (or xT [72, NSUB, SUB] for l=1).  Writes out_tile[:, m, :, :]."""
                dout = DIMS[l]
                mc = mc_l[l - 1]
                for m in range(mc):
                    pw = min(128, dout - 128 * m)
                    ps = pmm.tile([128, NSUB, SUB], f32, tag="mm")
                    for s in range(NSUB):
                        if l == 1:
                            nc.tensor.matmul(ps[:, s, :], w_sb[0],
                                             h_in[:, s, :], start=True, stop=True)
                        elif l == 2:
                            nc.tensor.matmul(
                                ps[:, s, :],
                                w_sb[1][:, 128 * m : 128 * m + pw],
                                h_in[:, 0, s, :], start=True, stop=True)
                        else:
                            kp = kp_l[l - 1]
                            for j in range(kp):
                                nc.tensor.matmul(
                                    ps[:pw, s, :],
                                    w_sb[l - 1][:, 2 * j : 2 * j + 2,
                                                128 * m : 128 * m + pw],
                                    h_in[:, 2 * j : 2 * j + 2, s, :],
                                    start=(j == 0), stop=(j == kp - 1),
                                    perf_mode=DR)
                    if out_l == "f1":
                        drain(l, m, ps[:pw], out_tile[0:72])
                    elif out_l == "f0":
                        drain(l, m, ps[:pw], out_tile)
                    else:
                        drain(l, m, ps[:pw], out_tile[:pw, m])

            def alloc_rec(t, xT):
                rec = {"t": t, "xT": xT}
                rec["f1"] = acts.tile([74, NSUB, SUB], bf16, tag=f"f1{t % 2}", name=f"f1_{t % 2}")
                # Pool-engine partition access must start 32-aligned; rows
                # 64:72 are re-written by the L7 m=1 drain below.
                nc.gpsimd.memset(rec["f1"][64:74], 1.0)
                for nm, kc, dt_h in (("h1", 1, bf16), ("h2", 2, f8),
                                     ("h3", 2, f8), ("h4", 4, f8),
                                     ("h5", 4, f8), ("h6", 4, f8)):
                    rec[nm] = acts.tile([128, kc, NSUB, SUB], dt_h,
                                        tag=f"{nm}{t % 2}", name=f"{nm}_{t % 2}")
                rec["f0"] = acts.tile([128, NSUB, SUB], bf16, tag=f"f0{t % 2}", name=f"f0_{t % 2}")
                return rec

            def layer(l, rec):
                if l < 7:
                    h_in = rec["xT"] if l == 1 else rec[f"h{l - 1}"]
                    mm_layer(l, h_in, None, rec[f"h{l}"], "h")
                    return
                # L7: m=0 -> f0, m=1 -> f1 rows 0:72
                dout, mc = DIMS[7], mc_l[6]
                for m in range(mc):
                    pw = min(128, dout - 128 * m)
                    ps = pmm.tile([128, NSUB, SUB], f32, tag="mm")
                    for s in range(NSUB):
                        for j in range(2):
                            nc.tensor.matmul(
                                ps[:pw, s, :],
                                w_sb[6][:, 2 * j : 2 * j + 2,
                                        128 * m : 128 * m + pw],
                                rec["h6"][:, 2 * j : 2 * j + 2, s, :],
                                start=(j == 0), stop=(j == 1), perf_mode=DR)
                    drain(7, m, ps[:pw], rec["f0"] if m == 0 else rec["f1"][0:72])

            def squares(rec):
                g0 = acts.tile([128, NSUB, SUB], bf16, tag=f"g0{rec['t'] % 2}")
                nc.gpsimd.tensor_tensor(out=g0, in0=rec["f0"], in1=rec["f0"],
                                        op=ALU.mult)
                g1 = acts.tile([72, NSUB, SUB], bf16, tag=f"g1{rec['t'] % 2}")
                nc.gpsimd.tensor_tensor(out=g1, in0=rec["f1"][0:72],
                                        in1=rec["f1"][0:72], op=ALU.mult)
                rec.update(g0=g0, g1=g1)

            # Two tiles are emitted interleaved layer-by-layer so one tile's
            # matmuls hide the other's PSUM-drain latency; the previous
            # pair's distance/normalize work is threaded between layers as
            # PE/DVE filler.
            assert NT % 2 == 0
            xts = [prefetch(0), prefetch(1)]
            prev = []
            for tp in range(0, NT, 2):
                a = alloc_rec(tp, xts[0])
                b = alloc_rec(tp + 1, xts[1])
                layer(1, a)
                layer(1, b)
                if tp + 2 < NT:
                    xts = [prefetch(tp + 2), prefetch(tp + 3)]
                if prev:
                    dist_pe(prev[0])
                    recip(prev[0])
                layer(2, a)
                layer(2, b)
                if prev:
                    tail(prev[0])
                layer(3, a)
                layer(3, b)
                if prev:
                    dist_pe(prev[1])
                    recip(prev[1])
                layer(4, a)
                layer(4, b)
                if prev:
                    tail(prev[1])
                layer(5, a)
                layer(5, b)
                layer(6, a)
                layer(6, b)
                layer(7, a)
                layer(7, b)
                squares(a)
                squares(b)
                prev = [a, b]
            for r in prev:
                dist_pe(r)
                recip(r)
                tail(r)

    nc.compile()
    return nc


def _prep_consts(ws, bs, center):
    """Host-side marshalling of the small replicated weights."""
    import ml_dtypes

    bf = ml_dtypes.bfloat16
    f8 = ml_dtypes.float8_e4m3
    consts = {}

    def q8(w, s):
        return np.clip(np.asarray(w, np.float32) * s, -240.0, 240.0).astype(f8)

    consts["w1"] = np.ascontiguousarray(np.asarray(ws[0], np.float32) * W1S).astype(bf)
    consts["w2"] = np.ascontiguousarray(np.asarray(ws[1], np.float32) * W2S).astype(bf)
    for l in range(3, 8):
        w = q8(ws[l - 1], S_W[l])  # [din, dout]
        din, dout = w.shape
        kc = din // 128
        if l == 7:
            wp = np.zeros((din, W7PAD), dtype=f8)
            wp[:, :dout] = w
            w, dout = wp, W7PAD
        consts[f"w{l}"] = np.ascontiguousarray(
            w.reshape(kc, 128, dout).transpose(1, 0, 2))
    mc_l = [1, 2, 2, 4, 4, 4, 2]
    for l in range(1, 8):
        dout = DIMS[l]
        bt = np.zeros((128, mc_l[l - 1]), dtype=np.float32)
        bias = np.asarray(bs[l - 1], np.float32) * A_ACT[l]
        for m in range(mc_l[l - 1]):
            pw = min(128, dout - 128 * m)
            bt[:pw, m] = bias[128 * m : 128 * m + pw]
        consts[f"b{l}"] = bt
    c = np.asarray(center, np.float64)
    consts["cm2A"] = np.ascontiguousarray(-2.0 * c[:128, :]).astype(np.float32).astype(bf)
    cm2B = np.zeros((74, 72), dtype=np.float32)
    cm2B[:72] = (-2.0 * c[128:200, :]).astype(np.float32)
    csq = (1.0 + (c ** 2).sum(axis=0)).astype(np.float32)
    csq_hi = csq.astype(bf).astype(np.float32)
    cm2B[72] = csq_hi          # hi part of csq (bf16-rounded)
    cm2B[73] = csq - csq_hi    # lo correction, small enough for bf16
    consts["cm2B"] = np.ascontiguousarray(cm2B).astype(bf)
    return consts


def _prep_inputs(inputs):
    """Full input dict -> per-core input maps (x transposed host-side)."""
    import ml_dtypes

    x = np.asarray(inputs["inputs"], np.float32)
    n = x.shape[0]
    n_loc = n // N_CORES
    xt = np.ascontiguousarray(x.T.astype(ml_dtypes.bfloat16))  # [72, N]
    consts = _prep_consts(
        [inputs[f"w{i}"] for i in range(1, 8)],
        [inputs[f"b{i}"] for i in range(1, 8)],
        inputs["center"],
    )
    in_maps = []
    for c in range(N_CORES):
        m = {"x": np.ascontiguousarray(xt[:, c * n_loc : (c + 1) * n_loc])}
        m.update(consts)
        in_maps.append(m)
    return in_maps, n_loc


def kernel(
    inputs, w1, b1, w2, b2, w3, b3, w4, b4, w5, b5, w6, b6, w7, b7, center
):
    from concourse.bass_utils import run_bass_kernel_spmd

    full = dict(inputs=inputs, w1=w1, b1=b1, w2=w2, b2=b2, w3=w3, b3=b3,
                w4=w4, b4=b4, w5=w5, b5=b5, w6=w6, b6=b6, w7=w7, b7=b7,
                center=center)
    in_maps, n_loc = _prep_inputs(full)
    if n_loc not in _CACHE:
        _CACHE[n_loc] = _build(n_loc)
    nc = _CACHE[n_loc]
    res = run_bass_kernel_spmd(nc, in_maps, core_ids=list(range(N_CORES)))
    return np.concatenate([res.results[c]["q"] for c in range(N_CORES)], axis=0)
